# revision 48
# baseline (speedup 1.0000x reference)
"""Trainium2 (Bass/Tile) kernel for the BiGNN layer problem — v18 final.

Self-contained: hardcodes N=50000, D=256, V=2, 8 NeuronCores.
Entry point: kernel(**inputs) -> np.ndarray [50000, 256] float32.

Math: for each of 4 edge sets s (bw_v0, bw_v1, fw_v0, fw_v1):
    h_s = relu( segsum_dst(x[src] @ W_s) / max(deg_dst,1) + b_s )   [N, 128]
  acc = [h_bw0 + h_bw1 | h_fw0 + h_fw1]                            [N, 256]
  out = acc @ W1 + b1 + x      (relu(acc)=acc since acc >= 0)

Design (measured ~1.55 ms on 8 cores vs 3.63 ms v2 baseline):
  - Dst-sharded across 8 cores (N/8 nodes each). Projection tables
    proj_s = x @ W_s computed shard-local, AllGather'd per set (set 0
    first so phase 2 starts while AG1-3 are in flight).
  - NON-transpose SWDGE dma_gather pulls proj rows EDGE-MAJOR
    [p=e%128, plane=e//128, 128f] straight into SBUF: no PE transpose,
    no PSUM->SBUF copy per slot. Tables split lo/hi at 32768 (int16
    idx); idx arrays padded with idx 0 to uniform x128 lengths.
  - Q7 descriptor generation is SERIAL per gather instruction on the
    Pool engine (~1.7ns/desc) and is the throughput limiter; 4 SWDGE
    queues give 4 descriptor rings (CH=1024 chunks, 48KB carveout) so
    the drain never stalls the generator. single_packet=True (1024 is
    the 64-desc/engine packet cap; False costs ~150us). Chunk counts
    set via reg_mov (one Pool op, not reg_alu's two).
  - Segment-sum as matmul: lhsT = one-hot O[e,d] (pure is_equal against
    an iota table, one DVE tensor_tensor per (set,tile)), rhs =
    gathered slot [e,f], accumulated over slots into PSUM S[d,f].
  - Epilogue (software-pipelined 2 units behind the matmuls):
    tensor_scalar_mul by per-partition 1/deg column (host-precomputed
    rmat), add bias row, ACT relu -> accS bf16; second set of each
    half added with one DVE add.
  - Phase 3 interleaved under set 3: PE-transpose accS halves, matmul
    against W1 halves, + b1 + x residual, DMA out.
  - Phase 1a batches x loads / converts / proj writes 4 tiles per DMA
    to cut Sync issue serialization in the lead-in.
"""

import math
from contextlib import ExitStack
from dataclasses import dataclass, field

import ml_dtypes
import numpy as np

import concourse.bacc as bacc
import concourse.bass as bass
import concourse.mybir as mybir
import concourse.tile as tile
from concourse.masks import make_identity

F32 = mybir.dt.float32
BF16 = mybir.dt.bfloat16
I16 = mybir.dt.int16
I32 = mybir.dt.int32
BF = ml_dtypes.bfloat16

T = 128     # dst rows per tile
CH = 1024   # max idxs per gather instruction
CARVEOUT = 49152  # descriptor ring: 3072 descs per queue per side
NQ = 4      # SWDGE queues


@dataclass
class Seg:
    n_idx: int = 0        # padded length (x128, uniform across cores)
    icol0: int = 0        # column offset in idx_all
    slot0: int = 0        # first slot in this tile's G
    min_cnt: int = 0      # min true count over cores (for pad memset)
    chunks: list = field(default_factory=list)  # of (offset, n, gidx)


@dataclass
class TilePlan:
    t: int = 0
    lo: Seg = field(default_factory=Seg)
    hi: Seg = field(default_factory=Seg)
    slots: int = 0
    dcol0: int = 0        # first dstf/redge column (== slot base)


@dataclass
class Plan:
    N: int
    NS: int
    n_cores: int
    n_tiles: int
    split: int
    IC: int = 0
    TS: int = 0
    NG: int = 0
    max_slots: int = 1
    set_icol0: list = field(default_factory=list)
    tiles: list = field(default_factory=list)      # [set][tile] -> TilePlan
    idx_all: list = field(default_factory=list)    # per core [128, IC] int16
    dstf_all: list = field(default_factory=list)   # per core [128, TS] bf16
    rmat_all: list = field(default_factory=list)   # per core [128, 4*n_tiles] f32


def make_plan(edge_sets, N, n_cores, split=32768):
    NS = N // n_cores
    assert NS * n_cores == N
    n_tiles = math.ceil(NS / T)
    S = len(edge_sets)
    plan = Plan(N=N, NS=NS, n_cores=n_cores, n_tiles=n_tiles, split=split)

    # bucket[s][c][t] = (lo_idx, lo_dmod, lo_r, hi_idx, hi_dmod, hi_r)
    buckets = [[[None] * n_tiles for _ in range(n_cores)] for _ in range(S)]
    rdegs = []
    for s, e in enumerate(edge_sets):
        src = np.asarray(e[0], dtype=np.int64)
        dst = np.asarray(e[1], dtype=np.int64)
        deg = np.bincount(dst, minlength=N).astype(np.float64)
        rdeg = (1.0 / np.maximum(deg, 1.0)).astype(np.float32)
        rdegs.append(rdeg)
        ishi = src >= split
        key = dst * 2 + ishi
        order = np.argsort(key, kind="stable")
        srcs, dsts, ishis = src[order], dst[order], ishi[order]
        rs = rdeg[dsts]
        keys = key[order]
        for c in range(n_cores):
            base = c * NS
            for t in range(n_tiles):
                r0, r1 = base + t * T, base + min((t + 1) * T, NS)
                i0 = np.searchsorted(keys, 2 * r0)
                i1 = np.searchsorted(keys, 2 * r1)
                seg_hi = ishis[i0:i1]
                lo_sel = ~seg_hi
                buckets[s][c][t] = (
                    srcs[i0:i1][lo_sel],
                    (dsts[i0:i1][lo_sel] - base) % T,
                    rs[i0:i1][lo_sel],
                    srcs[i0:i1][seg_hi] - split,
                    (dsts[i0:i1][seg_hi] - base) % T,
                    rs[i0:i1][seg_hi],
                )

    icol = dcol = gi = 0
    plan.set_icol0 = []
    for s in range(S):
        plan.set_icol0.append(icol)
        tl = []
        for t in range(n_tiles):
            tp = TilePlan(t=t)
            nl = max(len(buckets[s][c][t][0]) for c in range(n_cores))
            nh = max(len(buckets[s][c][t][3]) for c in range(n_cores))
            ml = max(min(len(buckets[s][c][t][0]) for c in range(n_cores)), 1)
            mh = max(min(len(buckets[s][c][t][3]) for c in range(n_cores)), 1)
            nl = ((nl + T - 1) // T) * T
            nh = ((nh + T - 1) // T) * T
            tp.lo = Seg(n_idx=nl, icol0=icol, slot0=0, min_cnt=min(ml, nl))
            icol += nl // 16
            tp.hi = Seg(n_idx=nh, icol0=icol, slot0=nl // T, min_cnt=min(mh, nh))
            icol += nh // 16
            tp.slots = (nl + nh) // T
            tp.dcol0 = dcol
            dcol += tp.slots
            for seg in (tp.lo, tp.hi):
                for o in range(0, seg.n_idx, CH):
                    seg.chunks.append((o, min(CH, seg.n_idx - o), gi))
                    gi += 1
            tl.append(tp)
        plan.tiles.append(tl)
    plan.IC = max(icol, 1)
    plan.TS = max(dcol, 1)
    plan.NG = max(gi, 1)
    plan.max_slots = max((tp.slots for tl in plan.tiles for tp in tl), default=1)

    for c in range(n_cores):
        idx_all = np.full((128, plan.IC), -1, dtype=np.int16)
        dstf_all = np.full((128, plan.TS), -1.0, dtype=BF)
        rmat = np.ones((128, S * n_tiles), dtype=np.float32)
        for s in range(S):
            for t in range(n_tiles):
                r0 = c * NS + t * T
                rows = min(T, NS - t * T)
                rmat[:rows, s * n_tiles + t] = rdegs[s][r0:r0 + rows]
        for s in range(S):
            for tp in plan.tiles[s]:
                b = buckets[s][c][tp.t]
                for seg, which in ((tp.lo, 0), (tp.hi, 3)):
                    if seg.n_idx == 0:
                        continue
                    bidx, bdm = b[which], b[which + 1]
                    nreal = len(bidx)
                    # pad idxs with -1: ucode trims trailing negatives, so
                    # this core emits descriptors only for its real edges
                    # pad with idx 0 (valid row; killed by dstf=-1 one-hot)
                    # so every plane is written and counts stay uniform
                    vals = np.zeros(seg.n_idx, dtype=np.int16)
                    vals[:nreal] = bidx.astype(np.int16)
                    dms = np.full(seg.n_idx, -1.0, dtype=np.float32)
                    dms[:nreal] = bdm
                    cols = seg.n_idx // 16
                    pat = vals.reshape(cols, 16).T
                    idx_all[:, seg.icol0:seg.icol0 + cols] = np.tile(pat, (8, 1))
                    dcols = seg.n_idx // T
                    d0 = tp.dcol0 + seg.slot0
                    dstf_all[:, d0:d0 + dcols] = dms.reshape(dcols, T).T.astype(BF)
        plan.idx_all.append(idx_all)
        plan.dstf_all.append(dstf_all)
        plan.rmat_all.append(rmat)
    return plan


def host_inputs(plan, c, x_full, Wall, ball, W1, b1):
    NS = plan.NS
    return {
        "x": np.ascontiguousarray(x_full[c * NS:(c + 1) * NS]).astype(np.float32),
        "Wall": Wall.astype(BF),
        "W1": W1.astype(BF),
        "ball": np.broadcast_to(ball.astype(np.float32), (128, 512)).copy(),
        "rmat": plan.rmat_all[c],
        "b1": np.broadcast_to(b1.astype(np.float32), (128, 256)).copy(),
        "iotab": np.broadcast_to(np.tile(np.arange(128, dtype=np.float32), plan.max_slots), (128, plan.max_slots * 128)).copy().astype(BF),
        "idx": plan.idx_all[c],
        "dstf": plan.dstf_all[c],
    }


def build_nc(plan):
    NS, n_tiles = plan.NS, plan.n_tiles
    n_cores = plan.n_cores
    D = 256
    max_slots = plan.max_slots
    max_icols = max((tp.lo.n_idx + tp.hi.n_idx) // 16 for tl in plan.tiles for tp in tl) if plan.tiles else 1

    nc = bacc.Bacc("TRN2", num_swdge_queues=NQ, dynamic_dma_scratch_size=CARVEOUT)
    x = nc.dram_tensor("x", [NS, D], F32, kind="ExternalInput")
    Wall = nc.dram_tensor("Wall", [D, 512], BF16, kind="ExternalInput")
    W1 = nc.dram_tensor("W1", [D, D], BF16, kind="ExternalInput")
    ball = nc.dram_tensor("ball", [128, 512], F32, kind="ExternalInput")
    rmat = nc.dram_tensor("rmat", [128, 4 * n_tiles], F32, kind="ExternalInput")
    b1 = nc.dram_tensor("b1", [128, D], F32, kind="ExternalInput")
    iotab = nc.dram_tensor("iotab", [128, max_slots * 128], BF16, kind="ExternalInput")
    idx = nc.dram_tensor("idx", [128, plan.IC], I16, kind="ExternalInput")
    dstf = nc.dram_tensor("dstf", [128, plan.TS], BF16, kind="ExternalInput")
    y = nc.dram_tensor("y", [NS, D], F32, kind="ExternalOutput")

    proj_local = [nc.dram_tensor(f"proj_local{s}", [NS, 128], BF16) for s in range(4)]
    tables = [nc.dram_tensor(f"table{s}", [plan.N, 128], BF16, addr_space="Shared")
              for s in range(4)]

    with tile.TileContext(nc) as tc, ExitStack() as ctx:
        const = ctx.enter_context(tc.tile_pool(name="const", bufs=1))
        sb = ctx.enter_context(tc.tile_pool(name="sb", bufs=5))
        p1 = ctx.enter_context(tc.tile_pool(name="p1", bufs=2))
        gpool = ctx.enter_context(tc.tile_pool(name="gp", bufs=5))
        opool = ctx.enter_context(tc.tile_pool(name="op", bufs=5))
        ipool = ctx.enter_context(tc.tile_pool(name="ip", bufs=2))
        ps_s = ctx.enter_context(tc.tile_pool(name="ps_s", bufs=4, space="PSUM"))
        ps_t = ctx.enter_context(tc.tile_pool(name="ps_t", bufs=2, space="PSUM"))
        ps_w = ctx.enter_context(tc.tile_pool(name="ps_w", bufs=2, space="PSUM"))

        # ---- constants ----
        wall_sb = []
        w1_sb = []
        for k in range(2):
            wt = const.tile([128, 512], BF16, name=f"wall{k}")
            nc.sync.dma_start(out=wt[:], in_=Wall[k * 128:(k + 1) * 128, :])
            wall_sb.append(wt)
            w1t = const.tile([128, D], BF16, name=f"w1_{k}")
            nc.sync.dma_start(out=w1t[:], in_=W1[k * 128:(k + 1) * 128, :])
            w1_sb.append(w1t)
        ball_sb = const.tile([128, 512], F32)
        nc.sync.dma_start(out=ball_sb[:], in_=ball[:])
        rmat_sb = const.tile([128, 4 * n_tiles], F32)
        nc.sync.dma_start(out=rmat_sb[:], in_=rmat[:])
        b1_sb = const.tile([128, D], F32)
        nc.sync.dma_start(out=b1_sb[:], in_=b1[:])
        iota_sb = const.tile([128, max_slots * 128], BF16)
        nc.sync.dma_start(out=iota_sb[:], in_=iotab[:])
        dstf_sb = const.tile([128, plan.TS], BF16)
        nc.sync.dma_start(out=dstf_sb[:], in_=dstf[:])
        idbf = const.tile([128, 128], BF16)
        make_identity(nc, idbf[:])
        # persistent transposed x (bf16) and acc (dst-major) tiles
        xT_all = const.tile([128, n_tiles * D], BF16)
        accS_all = const.tile([128, n_tiles * D], BF16)

        greg = ctx.enter_context(nc.gpsimd.register("gcnt_r"))

        # ---- phase 1a: x load + transpose + set-0 projection ----
        # groups of 4 tiles: one x DMA, one convert, one proj write
        GT = 4
        for g0 in range(0, n_tiles, GT):
            gts = list(range(g0, min(g0 + GT, n_tiles)))
            grows = NS - g0 * T if gts[-1] == n_tiles - 1 else len(gts) * T
            xt = p1.tile([128, GT, D], F32, tag="xt")
            nfull = grows // T  # whole 128-row planes
            if nfull:
                nc.sync.dma_start(
                    out=xt[:, :nfull, :],
                    in_=x[g0 * T:g0 * T + nfull * T, :].rearrange(
                        "(n p) d -> p n d", p=T),
                )
            rem = grows - nfull * T
            if rem:
                nc.sync.dma_start(out=xt[:rem, nfull, :],
                                  in_=x[g0 * T + nfull * T:g0 * T + grows, :])
            xb = p1.tile([128, GT, D], BF16, tag="xb")
            nlast = (grows + T - 1) // T
            nc.scalar.activation(out=xb[:, :nlast, :], in_=xt[:, :nlast, :],
                                 func=mybir.ActivationFunctionType.Copy)
            pb = p1.tile([128, GT, 128], BF16, tag="pb")
            for t in gts:
                rows = min(T, NS - t * T)
                j = t - g0
                for k in range(2):
                    tp = ps_t.tile([128, 128], BF16, tag="tp")
                    nc.tensor.transpose(
                        out=tp[:, :rows],
                        in_=xb[:rows, j, k * 128:(k + 1) * 128],
                        identity=idbf[:rows, :rows],
                    )
                    nc.vector.tensor_copy(
                        out=xT_all[:, t * D + k * 128:t * D + k * 128 + rows],
                        in_=tp[:, :rows])
                pp = ps_w.tile([128, 384], F32, tag="pp")
                for k in range(2):
                    nc.tensor.matmul(
                        out=pp[:rows, 0:128],
                        lhsT=xT_all[:, t * D + k * 128:t * D + k * 128 + rows],
                        rhs=wall_sb[k][:, 0:128],
                        start=(k == 0),
                        stop=(k == 1),
                    )
                nc.vector.tensor_copy(out=pb[:rows, j, :], in_=pp[:rows, 0:128])
            nfw = grows // T
            if nfw:
                nc.sync.dma_start(
                    out=proj_local[0][g0 * T:g0 * T + nfw * T, :].rearrange(
                        "(n p) d -> p n d", p=T),
                    in_=pb[:, :nfw, :])
            remw = grows - nfw * T
            if remw:
                nc.sync.dma_start(out=proj_local[0][g0 * T + nfw * T:g0 * T + grows, :],
                                  in_=pb[:remw, nfw, :])

        nc.gpsimd.collective_compute(
            "AllGather",
            mybir.AluOpType.bypass,
            replica_groups=[list(range(n_cores))],
            ins=[proj_local[0][:]],
            outs=[tables[0][:]],
        )

        # ---- phase 1b: projections for sets 1-3 (reuse xT) ----
        for t in range(n_tiles):
            rows = min(T, NS - t * T)
            pp = ps_w.tile([128, 384], F32, tag="pp")
            for k in range(2):
                nc.tensor.matmul(
                    out=pp[:rows, :],
                    lhsT=xT_all[:, t * D + k * 128:t * D + k * 128 + rows],
                    rhs=wall_sb[k][:, 128:512],
                    start=(k == 0),
                    stop=(k == 1),
                )
            pb = sb.tile([128, 384], BF16, tag="pb3")
            nc.vector.tensor_copy(out=pb[:rows, :], in_=pp[:rows, :])
            for s in range(1, 4):
                nc.sync.dma_start(out=proj_local[s][t * T:t * T + rows, :],
                                  in_=pb[:rows, (s - 1) * 128:s * 128])

        for s in range(1, 4):
            nc.gpsimd.collective_compute(
                "AllGather",
                mybir.AluOpType.bypass,
                replica_groups=[list(range(n_cores))],
                ins=[proj_local[s][:]],
                outs=[tables[s][:]],
            )

        # ---- phase 2: gather + one-hot segment sum (flipped matmul) ----
        set_cols = [(plan.set_icol0[s],
                     (plan.set_icol0[s + 1] if s < 3 else plan.IC) - plan.set_icol0[s])
                    for s in range(4)]
        max_set_cols = max(c for (_, c) in set_cols)
        ix_set = []
        for s in range(4):
            ix_t = ipool.tile([128, max_set_cols], I16, tag="ixset")
            nc.sync.dma_start(out=ix_t[:, :set_cols[s][1]],
                              in_=idx[:, set_cols[s][0]:set_cols[s][0] + set_cols[s][1]])
            ix_set.append(ix_t)
        qsel = 0
        pending_epi = []

        def emit_phase3(t):
            rows = min(T, NS - t * T)
            aTk = []
            for k in range(2):
                tp = ps_t.tile([128, 128], BF16, tag="tp")
                nc.tensor.transpose(
                    out=tp[:, :rows],
                    in_=accS_all[:rows, t * D + k * 128:t * D + (k + 1) * 128],
                    identity=idbf[:rows, :rows],
                )
                aT = sb.tile([128, 128], BF16, tag="aT")
                nc.scalar.activation(out=aT[:, :rows], in_=tp[:, :rows],
                                     func=mybir.ActivationFunctionType.Copy)
                aTk.append(aT)
            fpw = ps_w.tile([128, 384], F32, tag="pp")
            fp = fpw[:, 0:D]
            for k in range(2):
                nc.tensor.matmul(
                    out=fpw[:rows, 0:D],
                    lhsT=aTk[k][:, :rows],
                    rhs=w1_sb[k][:, :],
                    start=(k == 0),
                    stop=(k == 1),
                )
            xin = sb.tile([128, D], F32, tag="xt")
            nc.sync.dma_start(out=xin[:rows, :], in_=x[t * T:t * T + rows, :])
            ot = sb.tile([128, D], F32, tag="ot")
            nc.vector.tensor_tensor(out=ot[:rows, :], in0=fp[:rows, :],
                                    in1=b1_sb[:rows, :], op=mybir.AluOpType.add)
            nc.vector.tensor_tensor(out=ot[:rows, :], in0=ot[:rows, :],
                                    in1=xin[:rows, :], op=mybir.AluOpType.add)
            nc.sync.dma_start(out=y[t * T:t * T + rows, :], in_=ot[:rows, :])

        def emit_epilogue(s, t, Sp):
            half = 0 if s < 2 else 128
            accsl = accS_all[:, t * D + half:t * D + half + 128]
            tmp = sb.tile([128, 128], F32, tag="tmp")
            nc.vector.tensor_scalar_mul(out=tmp[:], in0=Sp[:],
                                        scalar1=rmat_sb[:, s * n_tiles + t:s * n_tiles + t + 1])
            nc.vector.tensor_tensor(out=tmp[:], in0=tmp[:],
                                    in1=ball_sb[:, s * 128:(s + 1) * 128],
                                    op=mybir.AluOpType.add)
            if s % 2 == 0:
                nc.scalar.activation(out=accsl, in_=tmp[:],
                                     func=mybir.ActivationFunctionType.Relu)
            else:
                tmp2 = sb.tile([128, 128], BF16, tag="tmp2")
                nc.scalar.activation(out=tmp2[:], in_=tmp[:],
                                     func=mybir.ActivationFunctionType.Relu)
                nc.vector.tensor_tensor(out=accsl, in0=accsl, in1=tmp2[:],
                                        op=mybir.AluOpType.add)
            if s == 3:
                emit_phase3(t)

        for s in range(4):
            ix_t = ix_set[s]
            sic0 = set_cols[s][0]
            for tp_ in plan.tiles[s]:
                t = tp_.t
                Sp = ps_s.tile([128, 128], F32, tag="S")
                if tp_.slots == 0:
                    nc.vector.memset(Sp[:], 0.0)
                else:
                    G3 = gpool.tile([128, max_slots, 128], BF16, tag="G")
                    for seg, tbl in ((tp_.lo, tables[s][0:plan.split, :]),
                                     (tp_.hi, tables[s][plan.split:plan.N, :])):
                        for (o, n, g) in seg.chunks:
                            nc.gpsimd.reg_mov(greg, n)
                            nc.gpsimd.dma_gather(
                                out_ap=G3[:, seg.slot0 + o // T:seg.slot0 + (o + n) // T, :],
                                in_ap=tbl,
                                idxs_ap=ix_t[:, seg.icol0 - sic0 + o // 16:seg.icol0 - sic0 + (o + n) // 16],
                                num_idxs=n,
                                num_idxs_reg=greg,
                                elem_size=128,
                                elem_step=128,
                                transpose=False,
                                single_packet=True,
                                queue_num=qsel,
                            )
                            qsel = (qsel + 1) % NQ
                    ns = tp_.slots
                    dc = tp_.dcol0
                    Ob = opool.tile([128, max_slots, 128], BF16, tag="O")
                    nh_ = (ns + 1) // 2
                    for (a, bnd) in ((0, nh_), (nh_, ns)):
                        if bnd > a:
                            nc.vector.tensor_tensor(
                                out=Ob[:, a:bnd, :],
                                in0=dstf_sb[:, dc + a:dc + bnd, None].to_broadcast([128, bnd - a, 128]),
                                in1=iota_sb[:, a * 128:bnd * 128],
                                op=mybir.AluOpType.is_equal,
                            )
                    for j in range(ns):
                        nc.tensor.matmul(out=Sp[:], lhsT=Ob[:, j, :],
                                         rhs=G3[:, j, :],
                                         start=(j == 0), stop=(j == ns - 1))
                # epilogue delayed 2 units so its cross-engine waits don't
                # head-of-line-block the DVE/ACT FIFOs for the next units
                pending_epi.append((s, t, Sp))
                if len(pending_epi) > 2:
                    emit_epilogue(*pending_epi.pop(0))
        for e in pending_epi:
            emit_epilogue(*e)

    nc.compile()
    return nc


def kernel(inps, fw_edges, bw_edges, W_fw, b_fw, W_bw, b_bw, W1, b1):
    """Full (unsharded) inputs in, full output out. Shards across 8 cores
    by destination node, runs the Bass kernel via run_bass_kernel_spmd."""
    from concourse.bass_utils import run_bass_kernel_spmd

    inps = np.asarray(inps)
    N = inps.shape[0]
    n_cores = 8
    Wall = np.concatenate([np.asarray(W_bw)[0], np.asarray(W_bw)[1],
                           np.asarray(W_fw)[0], np.asarray(W_fw)[1]], axis=1)
    ball = np.concatenate([np.asarray(b_bw)[0], np.asarray(b_bw)[1],
                           np.asarray(b_fw)[0], np.asarray(b_fw)[1]])
    edge_sets = [np.asarray(bw_edges)[0], np.asarray(bw_edges)[1],
                 np.asarray(fw_edges)[0], np.asarray(fw_edges)[1]]
    plan = make_plan(edge_sets, N, n_cores, split=32768)
    nc = build_nc(plan)
    in_maps = [host_inputs(plan, c, inps, Wall, ball, np.asarray(W1), np.asarray(b1))
               for c in range(n_cores)]
    res = run_bass_kernel_spmd(nc, in_maps, core_ids=list(range(n_cores)))
    out = np.concatenate([res.results[c]["y"] for c in range(n_cores)], axis=0)
    return out.astype(np.float32)


# revision 51
# speedup vs baseline: 1.0981x; 1.0981x over previous
"""Trainium2 (Bass/Tile) kernel for the BiGNN layer problem — v18 final.

Self-contained: hardcodes N=50000, D=256, V=2, 8 NeuronCores.
Entry point: kernel(**inputs) -> np.ndarray [50000, 256] float32.

Math: for each of 4 edge sets s (bw_v0, bw_v1, fw_v0, fw_v1):
    h_s = relu( segsum_dst(x[src] @ W_s) / max(deg_dst,1) + b_s )   [N, 128]
  acc = [h_bw0 + h_bw1 | h_fw0 + h_fw1]                            [N, 256]
  out = acc @ W1 + b1 + x      (relu(acc)=acc since acc >= 0)

Design (measured ~1.55 ms on 8 cores vs 3.63 ms v2 baseline):
  - Dst-sharded across 8 cores (N/8 nodes each). Projection tables
    proj_s = x @ W_s computed shard-local, AllGather'd per set (set 0
    first so phase 2 starts while AG1-3 are in flight).
  - NON-transpose SWDGE dma_gather pulls proj rows EDGE-MAJOR
    [p=e%128, plane=e//128, 128f] straight into SBUF: no PE transpose,
    no PSUM->SBUF copy per slot. Tables split lo/hi at 32768 (int16
    idx); idx arrays padded with idx 0 to uniform x128 lengths.
  - Q7 descriptor generation is SERIAL per gather instruction on the
    Pool engine (~1.7ns/desc) and is the throughput limiter; 4 SWDGE
    queues give 4 descriptor rings (CH=1024 chunks, 48KB carveout) so
    the drain never stalls the generator. single_packet=True (1024 is
    the 64-desc/engine packet cap; False costs ~150us). Chunk counts
    set via reg_mov (one Pool op, not reg_alu's two).
  - Segment-sum as matmul: lhsT = one-hot O[e,d] (pure is_equal against
    an iota table, one DVE tensor_tensor per (set,tile)), rhs =
    gathered slot [e,f], accumulated over slots into PSUM S[d,f].
  - Epilogue (software-pipelined 2 units behind the matmuls):
    tensor_scalar_mul by per-partition 1/deg column (host-precomputed
    rmat), add bias row, ACT relu -> accS bf16; second set of each
    half added with one DVE add.
  - Phase 3 interleaved under set 3: PE-transpose accS halves, matmul
    against W1 halves, + b1 + x residual, DMA out.
  - Phase 1a batches x loads / converts / proj writes 4 tiles per DMA
    to cut Sync issue serialization in the lead-in.
"""

import math
from contextlib import ExitStack
from dataclasses import dataclass, field

import ml_dtypes
import numpy as np

import concourse.bacc as bacc
import concourse.bass as bass
import concourse.mybir as mybir
import concourse.tile as tile
from concourse.masks import make_identity

F32 = mybir.dt.float32
BF16 = mybir.dt.bfloat16
I16 = mybir.dt.int16
I32 = mybir.dt.int32
BF = ml_dtypes.bfloat16

T = 128     # dst rows per tile
CH = 1024   # max idxs per gather instruction
CARVEOUT = 49152  # descriptor ring: 3072 descs per queue per side
NQ = 4      # SWDGE queues


@dataclass
class Seg:
    n_idx: int = 0        # padded length (x128, uniform across cores)
    icol0: int = 0        # column offset in idx_all
    slot0: int = 0        # first slot in this tile's G
    min_cnt: int = 0      # min true count over cores (for pad memset)
    chunks: list = field(default_factory=list)  # of (offset, n, gidx)


@dataclass
class TilePlan:
    t: int = 0
    lo: Seg = field(default_factory=Seg)
    hi: Seg = field(default_factory=Seg)
    slots: int = 0
    dcol0: int = 0        # first dstf/redge column (== slot base)


@dataclass
class Plan:
    N: int
    NS: int
    n_cores: int
    n_tiles: int
    split: int
    IC: int = 0
    TS: int = 0
    NG: int = 0
    max_slots: int = 1
    set_icol0: list = field(default_factory=list)
    tiles: list = field(default_factory=list)      # [set][tile] -> TilePlan
    idx_all: list = field(default_factory=list)    # per core [128, IC] int16
    dstf_all: list = field(default_factory=list)   # per core [128, TS] bf16
    rmat_all: list = field(default_factory=list)   # per core [128, 4*n_tiles] f32


def make_plan(edge_sets, N, n_cores, split=32768):
    NS = N // n_cores
    assert NS * n_cores == N
    n_tiles = math.ceil(NS / T)
    S = len(edge_sets)
    plan = Plan(N=N, NS=NS, n_cores=n_cores, n_tiles=n_tiles, split=split)

    # bucket[s][c][t] = (lo_idx, lo_dmod, lo_r, hi_idx, hi_dmod, hi_r)
    buckets = [[[None] * n_tiles for _ in range(n_cores)] for _ in range(S)]
    rdegs = []
    for s, e in enumerate(edge_sets):
        src = np.asarray(e[0], dtype=np.int64)
        dst = np.asarray(e[1], dtype=np.int64)
        deg = np.bincount(dst, minlength=N).astype(np.float64)
        rdeg = (1.0 / np.maximum(deg, 1.0)).astype(np.float32)
        rdegs.append(rdeg)
        ishi = src >= split
        key = dst * 2 + ishi
        order = np.argsort(key, kind="stable")
        srcs, dsts, ishis = src[order], dst[order], ishi[order]
        rs = rdeg[dsts]
        keys = key[order]
        for c in range(n_cores):
            base = c * NS
            for t in range(n_tiles):
                r0, r1 = base + t * T, base + min((t + 1) * T, NS)
                i0 = np.searchsorted(keys, 2 * r0)
                i1 = np.searchsorted(keys, 2 * r1)
                seg_hi = ishis[i0:i1]
                lo_sel = ~seg_hi
                buckets[s][c][t] = (
                    srcs[i0:i1][lo_sel],
                    (dsts[i0:i1][lo_sel] - base) % T,
                    rs[i0:i1][lo_sel],
                    srcs[i0:i1][seg_hi] - split,
                    (dsts[i0:i1][seg_hi] - base) % T,
                    rs[i0:i1][seg_hi],
                )

    icol = dcol = gi = 0
    plan.set_icol0 = []
    for s in range(S):
        plan.set_icol0.append(icol)
        tl = []
        for t in range(n_tiles):
            tp = TilePlan(t=t)
            nl = max(len(buckets[s][c][t][0]) for c in range(n_cores))
            nh = max(len(buckets[s][c][t][3]) for c in range(n_cores))
            ml = max(min(len(buckets[s][c][t][0]) for c in range(n_cores)), 1)
            mh = max(min(len(buckets[s][c][t][3]) for c in range(n_cores)), 1)
            nl = ((nl + T - 1) // T) * T
            nh = ((nh + T - 1) // T) * T
            tp.lo = Seg(n_idx=nl, icol0=icol, slot0=0, min_cnt=min(ml, nl))
            icol += nl // 16
            tp.hi = Seg(n_idx=nh, icol0=icol, slot0=nl // T, min_cnt=min(mh, nh))
            icol += nh // 16
            tp.slots = (nl + nh) // T
            tp.dcol0 = dcol
            dcol += tp.slots
            for seg in (tp.lo, tp.hi):
                for o in range(0, seg.n_idx, CH):
                    seg.chunks.append((o, min(CH, seg.n_idx - o), gi))
                    gi += 1
            tl.append(tp)
        plan.tiles.append(tl)
    plan.IC = max(icol, 1)
    plan.TS = max(dcol, 1)
    plan.NG = max(gi, 1)
    plan.max_slots = max((tp.slots for tl in plan.tiles for tp in tl), default=1)

    for c in range(n_cores):
        idx_all = np.full((128, plan.IC), -1, dtype=np.int16)
        dstf_all = np.full((128, plan.TS), -1.0, dtype=BF)
        rmat = np.ones((128, S * n_tiles), dtype=np.float32)
        for s in range(S):
            for t in range(n_tiles):
                r0 = c * NS + t * T
                rows = min(T, NS - t * T)
                rmat[:rows, s * n_tiles + t] = rdegs[s][r0:r0 + rows]
        for s in range(S):
            for tp in plan.tiles[s]:
                b = buckets[s][c][tp.t]
                for seg, which in ((tp.lo, 0), (tp.hi, 3)):
                    if seg.n_idx == 0:
                        continue
                    bidx, bdm = b[which], b[which + 1]
                    nreal = len(bidx)
                    # pad idxs with -1: ucode trims trailing negatives, so
                    # this core emits descriptors only for its real edges
                    # pad with idx 0 (valid row; killed by dstf=-1 one-hot)
                    # so every plane is written and counts stay uniform
                    vals = np.zeros(seg.n_idx, dtype=np.int16)
                    vals[:nreal] = bidx.astype(np.int16)
                    dms = np.full(seg.n_idx, -1.0, dtype=np.float32)
                    dms[:nreal] = bdm
                    cols = seg.n_idx // 16
                    pat = vals.reshape(cols, 16).T
                    idx_all[:, seg.icol0:seg.icol0 + cols] = np.tile(pat, (8, 1))
                    dcols = seg.n_idx // T
                    d0 = tp.dcol0 + seg.slot0
                    dstf_all[:, d0:d0 + dcols] = dms.reshape(dcols, T).T.astype(BF)
        plan.idx_all.append(idx_all)
        plan.dstf_all.append(dstf_all)
        plan.rmat_all.append(rmat)
    return plan


def host_inputs(plan, c, x_full, Wall, ball, W1, b1):
    NS = plan.NS
    return {
        "x": np.ascontiguousarray(x_full[c * NS:(c + 1) * NS]).astype(np.float32),
        "Wall": Wall.astype(BF),
        "W1": W1.astype(BF),
        "ball": np.broadcast_to(ball.astype(np.float32), (128, 512)).copy(),
        "rmat": plan.rmat_all[c],
        "b1": np.broadcast_to(b1.astype(np.float32), (128, 256)).copy(),
        "iotab": np.broadcast_to(np.tile(np.arange(128, dtype=np.float32), plan.max_slots), (128, plan.max_slots * 128)).copy().astype(BF),
        "idx": plan.idx_all[c],
        "dstf": plan.dstf_all[c],
    }


def build_nc(plan):
    NS, n_tiles = plan.NS, plan.n_tiles
    n_cores = plan.n_cores
    D = 256
    max_slots = plan.max_slots
    max_icols = max((tp.lo.n_idx + tp.hi.n_idx) // 16 for tl in plan.tiles for tp in tl) if plan.tiles else 1

    nc = bacc.Bacc("TRN2", num_swdge_queues=NQ, dynamic_dma_scratch_size=CARVEOUT)
    x = nc.dram_tensor("x", [NS, D], F32, kind="ExternalInput")
    Wall = nc.dram_tensor("Wall", [D, 512], BF16, kind="ExternalInput")
    W1 = nc.dram_tensor("W1", [D, D], BF16, kind="ExternalInput")
    ball = nc.dram_tensor("ball", [128, 512], F32, kind="ExternalInput")
    rmat = nc.dram_tensor("rmat", [128, 4 * n_tiles], F32, kind="ExternalInput")
    b1 = nc.dram_tensor("b1", [128, D], F32, kind="ExternalInput")
    iotab = nc.dram_tensor("iotab", [128, max_slots * 128], BF16, kind="ExternalInput")
    idx = nc.dram_tensor("idx", [128, plan.IC], I16, kind="ExternalInput")
    dstf = nc.dram_tensor("dstf", [128, plan.TS], BF16, kind="ExternalInput")
    y = nc.dram_tensor("y", [NS, D], F32, kind="ExternalOutput")

    proj_local = [nc.dram_tensor(f"proj_local{s}", [NS, 128], BF16) for s in range(4)]
    tables = [nc.dram_tensor(f"table{s}", [plan.N, 128], BF16, addr_space="Shared")
              for s in range(4)]

    with tile.TileContext(nc) as tc, ExitStack() as ctx:
        const = ctx.enter_context(tc.tile_pool(name="const", bufs=1))
        sb = ctx.enter_context(tc.tile_pool(name="sb", bufs=5))
        p1 = ctx.enter_context(tc.tile_pool(name="p1", bufs=2))
        gpool = ctx.enter_context(tc.tile_pool(name="gp", bufs=5))
        opool = ctx.enter_context(tc.tile_pool(name="op", bufs=5))
        ipool = ctx.enter_context(tc.tile_pool(name="ip", bufs=2))
        ps_s = ctx.enter_context(tc.tile_pool(name="ps_s", bufs=4, space="PSUM"))
        ps_t = ctx.enter_context(tc.tile_pool(name="ps_t", bufs=2, space="PSUM"))
        ps_w = ctx.enter_context(tc.tile_pool(name="ps_w", bufs=2, space="PSUM"))

        # ---- constants ----
        wall_sb = []
        w1_sb = []
        for k in range(2):
            wt = const.tile([128, 512], BF16, name=f"wall{k}")
            nc.sync.dma_start(out=wt[:], in_=Wall[k * 128:(k + 1) * 128, :])
            wall_sb.append(wt)
            w1t = const.tile([128, D], BF16, name=f"w1_{k}")
            nc.sync.dma_start(out=w1t[:], in_=W1[k * 128:(k + 1) * 128, :])
            w1_sb.append(w1t)
        ball_sb = const.tile([128, 512], F32)
        nc.sync.dma_start(out=ball_sb[:], in_=ball[:])
        rmat_sb = const.tile([128, 4 * n_tiles], F32)
        nc.sync.dma_start(out=rmat_sb[:], in_=rmat[:])
        b1_sb = const.tile([128, D], F32)
        nc.sync.dma_start(out=b1_sb[:], in_=b1[:])
        iota_sb = const.tile([128, max_slots * 128], BF16)
        nc.sync.dma_start(out=iota_sb[:], in_=iotab[:])
        dstf_sb = const.tile([128, plan.TS], BF16)
        nc.sync.dma_start(out=dstf_sb[:], in_=dstf[:])
        idbf = const.tile([128, 128], BF16)
        make_identity(nc, idbf[:])
        # persistent transposed x (bf16) and acc (dst-major) tiles
        xT_all = const.tile([128, n_tiles * D], BF16)
        accS_all = const.tile([128, n_tiles * D], BF16)

        greg = ctx.enter_context(nc.gpsimd.register("gcnt_r"))

        # ---- phase 1a: x load + transpose + set-0 projection ----
        # groups of 4 tiles: one x DMA, one convert, one proj write
        GT = 4
        for g0 in range(0, n_tiles, GT):
            gts = list(range(g0, min(g0 + GT, n_tiles)))
            grows = NS - g0 * T if gts[-1] == n_tiles - 1 else len(gts) * T
            xt = p1.tile([128, GT, D], F32, tag="xt")
            nfull = grows // T  # whole 128-row planes
            if nfull:
                nc.sync.dma_start(
                    out=xt[:, :nfull, :],
                    in_=x[g0 * T:g0 * T + nfull * T, :].rearrange(
                        "(n p) d -> p n d", p=T),
                )
            rem = grows - nfull * T
            if rem:
                nc.sync.dma_start(out=xt[:rem, nfull, :],
                                  in_=x[g0 * T + nfull * T:g0 * T + grows, :])
            xb = p1.tile([128, GT, D], BF16, tag="xb")
            nlast = (grows + T - 1) // T
            nc.scalar.activation(out=xb[:, :nlast, :], in_=xt[:, :nlast, :],
                                 func=mybir.ActivationFunctionType.Copy)
            pb = p1.tile([128, GT, 128], BF16, tag="pb")
            for t in gts:
                rows = min(T, NS - t * T)
                j = t - g0
                for k in range(2):
                    tp = ps_t.tile([128, 128], BF16, tag="tp")
                    nc.tensor.transpose(
                        out=tp[:, :rows],
                        in_=xb[:rows, j, k * 128:(k + 1) * 128],
                        identity=idbf[:rows, :rows],
                    )
                    nc.vector.tensor_copy(
                        out=xT_all[:, t * D + k * 128:t * D + k * 128 + rows],
                        in_=tp[:, :rows])
                pp = ps_w.tile([128, 384], F32, tag="pp")
                for k in range(2):
                    nc.tensor.matmul(
                        out=pp[:rows, 0:128],
                        lhsT=xT_all[:, t * D + k * 128:t * D + k * 128 + rows],
                        rhs=wall_sb[k][:, 0:128],
                        start=(k == 0),
                        stop=(k == 1),
                    )
                nc.vector.tensor_copy(out=pb[:rows, j, :], in_=pp[:rows, 0:128])
            nfw = grows // T
            if nfw:
                nc.sync.dma_start(
                    out=proj_local[0][g0 * T:g0 * T + nfw * T, :].rearrange(
                        "(n p) d -> p n d", p=T),
                    in_=pb[:, :nfw, :])
            remw = grows - nfw * T
            if remw:
                nc.sync.dma_start(out=proj_local[0][g0 * T + nfw * T:g0 * T + grows, :],
                                  in_=pb[:remw, nfw, :])

        nc.gpsimd.collective_compute(
            "AllGather",
            mybir.AluOpType.bypass,
            replica_groups=[list(range(n_cores))],
            ins=[proj_local[0][:]],
            outs=[tables[0][:]],
        )

        # ---- phase 1b: projections for sets 1-3 (reuse xT) ----
        for t in range(n_tiles):
            rows = min(T, NS - t * T)
            pp = ps_w.tile([128, 384], F32, tag="pp")
            for k in range(2):
                nc.tensor.matmul(
                    out=pp[:rows, :],
                    lhsT=xT_all[:, t * D + k * 128:t * D + k * 128 + rows],
                    rhs=wall_sb[k][:, 128:512],
                    start=(k == 0),
                    stop=(k == 1),
                )
            pb = sb.tile([128, 384], BF16, tag="pb3")
            nc.vector.tensor_copy(out=pb[:rows, :], in_=pp[:rows, :])
            for s in range(1, 4):
                nc.sync.dma_start(out=proj_local[s][t * T:t * T + rows, :],
                                  in_=pb[:rows, (s - 1) * 128:s * 128])

        for s in range(1, 4):
            nc.gpsimd.collective_compute(
                "AllGather",
                mybir.AluOpType.bypass,
                replica_groups=[list(range(n_cores))],
                ins=[proj_local[s][:]],
                outs=[tables[s][:]],
            )

        # ---- phase 2: gather + one-hot segment sum (flipped matmul) ----
        set_cols = [(plan.set_icol0[s],
                     (plan.set_icol0[s + 1] if s < 3 else plan.IC) - plan.set_icol0[s])
                    for s in range(4)]
        max_set_cols = max(c for (_, c) in set_cols)
        ix_set = []
        for s in range(4):
            ix_t = ipool.tile([128, max_set_cols], I16, tag="ixset")
            nc.sync.dma_start(out=ix_t[:, :set_cols[s][1]],
                              in_=idx[:, set_cols[s][0]:set_cols[s][0] + set_cols[s][1]])
            ix_set.append(ix_t)
        qsel = 0
        pending_epi = []

        def emit_phase3(t):
            rows = min(T, NS - t * T)
            aTk = []
            for k in range(2):
                tp = ps_t.tile([128, 128], BF16, tag="tp")
                nc.tensor.transpose(
                    out=tp[:, :rows],
                    in_=accS_all[:rows, t * D + k * 128:t * D + (k + 1) * 128],
                    identity=idbf[:rows, :rows],
                )
                aT = sb.tile([128, 128], BF16, tag="aT")
                nc.scalar.activation(out=aT[:, :rows], in_=tp[:, :rows],
                                     func=mybir.ActivationFunctionType.Copy)
                aTk.append(aT)
            fpw = ps_w.tile([128, 384], F32, tag="pp")
            fp = fpw[:, 0:D]
            for k in range(2):
                nc.tensor.matmul(
                    out=fpw[:rows, 0:D],
                    lhsT=aTk[k][:, :rows],
                    rhs=w1_sb[k][:, :],
                    start=(k == 0),
                    stop=(k == 1),
                )
            xin = sb.tile([128, D], F32, tag="xt")
            nc.sync.dma_start(out=xin[:rows, :], in_=x[t * T:t * T + rows, :])
            ot = sb.tile([128, D], F32, tag="ot")
            nc.vector.tensor_tensor(out=ot[:rows, :], in0=fp[:rows, :],
                                    in1=b1_sb[:rows, :], op=mybir.AluOpType.add)
            nc.vector.tensor_tensor(out=ot[:rows, :], in0=ot[:rows, :],
                                    in1=xin[:rows, :], op=mybir.AluOpType.add)
            nc.sync.dma_start(out=y[t * T:t * T + rows, :], in_=ot[:rows, :])

        def emit_epilogue(s, t, Sp):
            half = 0 if s < 2 else 128
            accsl = accS_all[:, t * D + half:t * D + half + 128]
            tmp = sb.tile([128, 128], F32, tag="tmp")
            nc.vector.tensor_scalar_mul(out=tmp[:], in0=Sp[:],
                                        scalar1=rmat_sb[:, s * n_tiles + t:s * n_tiles + t + 1])
            nc.vector.tensor_tensor(out=tmp[:], in0=tmp[:],
                                    in1=ball_sb[:, s * 128:(s + 1) * 128],
                                    op=mybir.AluOpType.add)
            if s % 2 == 0:
                nc.scalar.activation(out=accsl, in_=tmp[:],
                                     func=mybir.ActivationFunctionType.Relu)
            else:
                tmp2 = sb.tile([128, 128], BF16, tag="tmp2")
                nc.scalar.activation(out=tmp2[:], in_=tmp[:],
                                     func=mybir.ActivationFunctionType.Relu)
                nc.vector.tensor_tensor(out=accsl, in0=accsl, in1=tmp2[:],
                                        op=mybir.AluOpType.add)
            if s == 3:
                emit_phase3(t)

        for s in range(4):
            ix_t = ix_set[s]
            sic0 = set_cols[s][0]
            for tp_ in plan.tiles[s]:
                t = tp_.t
                Sp = ps_s.tile([128, 128], F32, tag="S")
                if tp_.slots == 0:
                    nc.vector.memset(Sp[:], 0.0)
                else:
                    G3 = gpool.tile([128, max_slots, 128], BF16, tag="G")
                    for seg, tbl in ((tp_.lo, tables[s][0:plan.split, :]),
                                     (tp_.hi, tables[s][plan.split:plan.N, :])):
                        for (o, n, g) in seg.chunks:
                            nc.gpsimd.reg_mov(greg, n)
                            nc.gpsimd.dma_gather(
                                out_ap=G3[:, seg.slot0 + o // T:seg.slot0 + (o + n) // T, :],
                                in_ap=tbl,
                                idxs_ap=ix_t[:, seg.icol0 - sic0 + o // 16:seg.icol0 - sic0 + (o + n) // 16],
                                num_idxs=n,
                                num_idxs_reg=greg,
                                elem_size=128,
                                elem_step=128,
                                transpose=False,
                                single_packet=True,
                                queue_num=qsel,
                            )
                            qsel = (qsel + 1) % NQ
                    ns = tp_.slots
                    dc = tp_.dcol0
                    Ob = opool.tile([128, max_slots, 128], BF16, tag="O")
                    nc.vector.tensor_tensor(
                        out=Ob[:, :ns, :],
                        in0=dstf_sb[:, dc:dc + ns, None].to_broadcast([128, ns, 128]),
                        in1=iota_sb[:, :ns * 128],
                        op=mybir.AluOpType.is_equal,
                    )
                    for j in range(ns):
                        nc.tensor.matmul(out=Sp[:], lhsT=Ob[:, j, :],
                                         rhs=G3[:, j, :],
                                         start=(j == 0), stop=(j == ns - 1))
                # epilogue delayed 2 units so its cross-engine waits don't
                # head-of-line-block the DVE/ACT FIFOs for the next units
                pending_epi.append((s, t, Sp))
                if len(pending_epi) > 2:
                    emit_epilogue(*pending_epi.pop(0))
        for e in pending_epi:
            emit_epilogue(*e)

    nc.compile()
    return nc


def kernel(inps, fw_edges, bw_edges, W_fw, b_fw, W_bw, b_bw, W1, b1):
    """Full (unsharded) inputs in, full output out. Shards across 8 cores
    by destination node, runs the Bass kernel via run_bass_kernel_spmd."""
    from concourse.bass_utils import run_bass_kernel_spmd

    inps = np.asarray(inps)
    N = inps.shape[0]
    n_cores = 8
    Wall = np.concatenate([np.asarray(W_bw)[0], np.asarray(W_bw)[1],
                           np.asarray(W_fw)[0], np.asarray(W_fw)[1]], axis=1)
    ball = np.concatenate([np.asarray(b_bw)[0], np.asarray(b_bw)[1],
                           np.asarray(b_fw)[0], np.asarray(b_fw)[1]])
    edge_sets = [np.asarray(bw_edges)[0], np.asarray(bw_edges)[1],
                 np.asarray(fw_edges)[0], np.asarray(fw_edges)[1]]
    plan = make_plan(edge_sets, N, n_cores, split=32768)
    nc = build_nc(plan)
    in_maps = [host_inputs(plan, c, inps, Wall, ball, np.asarray(W1), np.asarray(b1))
               for c in range(n_cores)]
    res = run_bass_kernel_spmd(nc, in_maps, core_ids=list(range(n_cores)))
    out = np.concatenate([res.results[c]["y"] for c in range(n_cores)], axis=0)
    return out.astype(np.float32)
